# revision 4
# baseline (speedup 1.0000x reference)
"""CenterLoss kernel for 8 Trainium2 NeuronCores (data-parallel over batch).

loss = ( sum_b clip(||x_b - centers[labels_b]||^2, 1e-12, 1e12)
         + (B*C - B)*1e-12 ) / B

Per core (128 batch rows): labels -> SBUF, then 4 column-chunked indirect
DMA gathers (raw labels as row indices, element_offset selects the column
slice) pipelined against 4 x-chunk loads; DVE subtracts per chunk while
ACT squares+row-accumulates each chunk behind it; DVE reduces the 4
partial columns + clips; PE sums partitions via a ones matmul; SP
register-load/stores the scalar to DRAM (no output DMA). An all-engine
barrier + semaphore clear at the end makes the NEFF safe to re-execute.
"""

import sys

if "/opt/trn_rl_repo" not in sys.path:
    sys.path.insert(0, "/opt/trn_rl_repo")

import numpy as np

import concourse.bass as bass
import concourse.mybir as mybir
from concourse.bass_utils import run_bass_kernel_spmd

B = 1024
C = 8192
D = 2048
N_CORES = 8
P = B // N_CORES  # 128
CHUNK_WIDTHS = [512, 512, 512, 512]
NCHUNK = len(CHUNK_WIDTHS)
CHUNK_STARTS = [sum(CHUNK_WIDTHS[:i]) for i in range(NCHUNK)]
assert sum(CHUNK_WIDTHS) == D

_CACHE: dict = {}


def _build():
    f32 = mybir.dt.float32
    i32 = mybir.dt.int32

    nc = bass.Bass("TRN2", target_bir_lowering=False, debug=False, num_devices=N_CORES)
    x_ap = nc.dram_tensor("x", (P, D), f32, kind="ExternalInput").ap()
    lab_ap = nc.dram_tensor("labels", (P, 1), i32, kind="ExternalInput").ap()
    cen_ap = nc.dram_tensor("centers", (C, D), f32, kind="ExternalInput").ap()
    out_ap = nc.dram_tensor("out", (1, 1), f32, kind="ExternalOutput").ap()

    ones = nc.const_aps.tensor(1.0, (P, 1), f32)

    from contextlib import ExitStack

    with ExitStack() as ctx:
        x_t = ctx.enter_context(nc.sbuf_tensor("x_t", [P, D], f32))
        cg_t = ctx.enter_context(nc.sbuf_tensor("cg_t", [P, D], f32))
        diff_t = ctx.enter_context(nc.sbuf_tensor("diff_t", [P, D], f32))
        lab_t = ctx.enter_context(nc.sbuf_tensor("lab_t", [P, 1], i32))
        dist4_t = ctx.enter_context(nc.sbuf_tensor("dist4_t", [P, NCHUNK], f32))
        dist_t = ctx.enter_context(nc.sbuf_tensor("dist_t", [P, 1], f32))
        res_t = ctx.enter_context(nc.sbuf_tensor("res_t", [1, 1], f32))
        psum_t = ctx.enter_context(nc.psum_tensor("psum_t", [1, 1], f32))

        lab_sem = ctx.enter_context(nc.semaphore("lab_sem"))
        x_sems = [ctx.enter_context(nc.semaphore(f"x_sem{i}")) for i in range(NCHUNK)]
        g_sems = [ctx.enter_context(nc.semaphore(f"g_sem{i}")) for i in range(NCHUNK)]
        v_sem = ctx.enter_context(nc.semaphore("v_sem"))
        a_sem = ctx.enter_context(nc.semaphore("a_sem"))
        r_sem = ctx.enter_context(nc.semaphore("r_sem"))
        mm_sem = ctx.enter_context(nc.semaphore("mm_sem"))
        done_sem = ctx.enter_context(nc.semaphore("done_sem"))
        block = ctx.enter_context(nc.Block())

        sems = [lab_sem, *x_sems, *g_sems, v_sem, a_sem, r_sem, mm_sem,
                done_sem]
        sem_nums = sorted(s.num for s in sems)
        assert sem_nums == list(range(sem_nums[0], sem_nums[0] + len(sems)))
        sem_range = range(sem_nums[0], sem_nums[-1] + 1)

        def cols(c):
            return slice(CHUNK_STARTS[c], CHUNK_STARTS[c] + CHUNK_WIDTHS[c])

        @block.sync
        def _(sync):
            sync.dma_start(out=lab_t[:], in_=lab_ap[:]).then_inc(lab_sem, 16)
            for c in range(NCHUNK):
                sync.dma_start(out=x_t[:, cols(c)], in_=x_ap[:, cols(c)]).then_inc(
                    x_sems[c], 16
                )
            sync.wait_ge(done_sem, 1)
            reg = nc.sync.alloc_register()
            sync.load(reg, res_t[0:1, 0:1].bitcast(i32))
            sync.store(out_ap[0:1, 0:1].bitcast(i32), reg)

        @block.gpsimd
        def _(gpsimd):
            gpsimd.wait_ge(lab_sem, 16)
            for c in range(NCHUNK):
                gpsimd.indirect_dma_start(
                    out=cg_t[:, cols(c)],
                    out_offset=None,
                    in_=cen_ap[:],
                    in_offset=bass.IndirectOffsetOnAxis(ap=lab_t[:, :1], axis=0),
                    element_offset=CHUNK_STARTS[c],
                ).then_inc(g_sems[c], 16)


        @block.vector
        def _(vector):
            for c in range(NCHUNK):
                vector.wait_ge(x_sems[c], 16)
                vector.wait_ge(g_sems[c], 16)
                nc.vector.tensor_tensor(
                    out=diff_t[:, cols(c)],
                    in0=x_t[:, cols(c)],
                    in1=cg_t[:, cols(c)],
                    op=mybir.AluOpType.subtract,
                ).then_inc(v_sem, 1)
            # DVE is pipelined, so same-engine RAW chains need explicit waits.
            vector.wait_ge(a_sem, NCHUNK)
            nc.vector.reduce_sum(
                out=dist_t[:], in_=dist4_t[:], axis=mybir.AxisListType.X
            ).then_inc(v_sem, 1)
            vector.wait_ge(v_sem, NCHUNK + 1)
            nc.vector.tensor_scalar(
                out=dist_t[:],
                in0=dist_t[:],
                scalar1=1e-12,
                scalar2=1e12,
                op0=mybir.AluOpType.max,
                op1=mybir.AluOpType.min,
            ).then_inc(r_sem, 1)
            vector.wait_ge(mm_sem, 1)
            nc.vector.tensor_copy(out=res_t[:], in_=psum_t[:]).then_inc(done_sem, 1)

        @block.scalar
        def _(scalar):
            for c in range(NCHUNK):
                scalar.wait_ge(v_sem, c + 1)
                nc.scalar.activation(
                    out=x_t[:, cols(c)],
                    in_=diff_t[:, cols(c)],
                    func=mybir.ActivationFunctionType.Square,
                    accum_out=dist4_t[:, c : c + 1],
                ).then_inc(a_sem, 1)


        @block.tensor
        def _(tensor):
            tensor.wait_ge(r_sem, 1)
            nc.tensor.matmul(
                out=psum_t[:], lhsT=dist_t[:], rhs=ones, start=True, stop=True
            ).then_inc(mm_sem, 1)

        # Re-execution safety: the same loaded NEFF runs many times, so all
        # kernel sems must end at 0. Builtin all-engine barrier (self-
        # resetting gather/release sems) orders every engine's updates
        # before Pool drains DMA state and zeroes the kernel semaphores.
        nc.all_engine_barrier()
        nc.gpsimd.dma_reset(sem_range)
        nc.gpsimd.sem_clear(sem_range)

    return nc


def _get_nc():
    if "nc" not in _CACHE:
        _CACHE["nc"] = _build()
    return _CACHE["nc"]


def kernel(x: np.ndarray, labels: np.ndarray, centers: np.ndarray) -> np.ndarray:
    x = np.ascontiguousarray(np.asarray(x, dtype=np.float32))
    centers = np.ascontiguousarray(np.asarray(centers, dtype=np.float32))
    lab = np.asarray(labels).astype(np.int32).reshape(B, 1)

    nc = _get_nc()
    in_maps = []
    for c in range(N_CORES):
        sl = slice(c * P, (c + 1) * P)
        in_maps.append(
            {
                "x": np.ascontiguousarray(x[sl]),
                "labels": np.ascontiguousarray(lab[sl]),
                "centers": centers,
            }
        )
    res = run_bass_kernel_spmd(nc, in_maps, list(range(N_CORES)))

    total = 0.0
    for c in range(N_CORES):
        total += float(res.results[c]["out"][0, 0])
    total += (B * C - B) * 1e-12
    return np.float32(total / B)


# revision 5
# speedup vs baseline: 1.0150x; 1.0150x over previous
"""CenterLoss kernel for 8 Trainium2 NeuronCores (data-parallel over batch).

loss = ( sum_b clip(||x_b - centers[labels_b]||^2, 1e-12, 1e12)
         + (B*C - B)*1e-12 ) / B

Per core (128 batch rows): labels -> SBUF, then 4 column-chunked indirect
DMA gathers (raw labels as row indices, element_offset selects the column
slice) pipelined against 4 x-chunk loads; DVE subtracts per chunk while
ACT squares+row-accumulates each chunk behind it; DVE reduces the 4
partial columns + clips; PE sums partitions via a ones matmul; SP
register-load/stores the scalar to DRAM (no output DMA). An all-engine
barrier + semaphore clear at the end makes the NEFF safe to re-execute.
"""

import sys

if "/opt/trn_rl_repo" not in sys.path:
    sys.path.insert(0, "/opt/trn_rl_repo")

import numpy as np

import concourse.bass as bass
import concourse.mybir as mybir
from concourse.bass_utils import run_bass_kernel_spmd

B = 1024
C = 8192
D = 2048
N_CORES = 8
P = B // N_CORES  # 128
CHUNK_WIDTHS = [512, 512, 512, 512]
NCHUNK = len(CHUNK_WIDTHS)
CHUNK_STARTS = [sum(CHUNK_WIDTHS[:i]) for i in range(NCHUNK)]
assert sum(CHUNK_WIDTHS) == D

_CACHE: dict = {}


def _build():
    f32 = mybir.dt.float32
    i32 = mybir.dt.int32

    nc = bass.Bass("TRN2", target_bir_lowering=False, debug=False, num_devices=N_CORES)
    # Drop the two preamble const memsets this kernel never reads (bf16 1.0,
    # uint8 127) — they serialize on Pool ahead of the preamble barrier and
    # delay the first DMA.
    _bb = nc.cur_bb.bb
    for _ins in [
        i
        for i in _bb.instructions
        if type(i).__name__ in ("InstMemSet", "InstMemset")
        and ("bfloat16" in str(i) or "uint8" in str(i))
    ]:
        _bb.instructions.remove(_ins)

    x_ap = nc.dram_tensor("x", (P, D), f32, kind="ExternalInput").ap()
    lab_ap = nc.dram_tensor("labels", (P, 1), i32, kind="ExternalInput").ap()
    cen_ap = nc.dram_tensor("centers", (C, D), f32, kind="ExternalInput").ap()
    out_ap = nc.dram_tensor("out", (1, 1), f32, kind="ExternalOutput").ap()

    ones = nc.const_aps.tensor(1.0, (P, 1), f32)

    from contextlib import ExitStack

    with ExitStack() as ctx:
        x_t = ctx.enter_context(nc.sbuf_tensor("x_t", [P, D], f32))
        cg_t = ctx.enter_context(nc.sbuf_tensor("cg_t", [P, D], f32))
        diff_t = ctx.enter_context(nc.sbuf_tensor("diff_t", [P, D], f32))
        lab_t = ctx.enter_context(nc.sbuf_tensor("lab_t", [P, 1], i32))
        dist4_t = ctx.enter_context(nc.sbuf_tensor("dist4_t", [P, NCHUNK], f32))
        dist_t = ctx.enter_context(nc.sbuf_tensor("dist_t", [P, 1], f32))
        res_t = ctx.enter_context(nc.sbuf_tensor("res_t", [1, 1], f32))
        psum_t = ctx.enter_context(nc.psum_tensor("psum_t", [1, 1], f32))

        lab_sem = ctx.enter_context(nc.semaphore("lab_sem"))
        x_sems = [ctx.enter_context(nc.semaphore(f"x_sem{i}")) for i in range(NCHUNK)]
        g_sems = [ctx.enter_context(nc.semaphore(f"g_sem{i}")) for i in range(NCHUNK)]
        v_sem = ctx.enter_context(nc.semaphore("v_sem"))
        a_sem = ctx.enter_context(nc.semaphore("a_sem"))
        r_sem = ctx.enter_context(nc.semaphore("r_sem"))
        mm_sem = ctx.enter_context(nc.semaphore("mm_sem"))
        done_sem = ctx.enter_context(nc.semaphore("done_sem"))
        block = ctx.enter_context(nc.Block())

        sems = [lab_sem, *x_sems, *g_sems, v_sem, a_sem, r_sem, mm_sem,
                done_sem]
        sem_nums = sorted(s.num for s in sems)
        assert sem_nums == list(range(sem_nums[0], sem_nums[0] + len(sems)))
        sem_range = range(sem_nums[0], sem_nums[-1] + 1)

        def cols(c):
            return slice(CHUNK_STARTS[c], CHUNK_STARTS[c] + CHUNK_WIDTHS[c])

        @block.sync
        def _(sync):
            sync.dma_start(out=lab_t[:], in_=lab_ap[:]).then_inc(lab_sem, 16)
            for c in range(NCHUNK):
                sync.dma_start(out=x_t[:, cols(c)], in_=x_ap[:, cols(c)]).then_inc(
                    x_sems[c], 16
                )
            sync.wait_ge(done_sem, 1)
            reg = nc.sync.alloc_register()
            sync.load(reg, res_t[0:1, 0:1].bitcast(i32))
            sync.store(out_ap[0:1, 0:1].bitcast(i32), reg)

        @block.gpsimd
        def _(gpsimd):
            gpsimd.wait_ge(lab_sem, 16)
            for c in range(NCHUNK):
                gpsimd.indirect_dma_start(
                    out=cg_t[:, cols(c)],
                    out_offset=None,
                    in_=cen_ap[:],
                    in_offset=bass.IndirectOffsetOnAxis(ap=lab_t[:, :1], axis=0),
                    element_offset=CHUNK_STARTS[c],
                ).then_inc(g_sems[c], 16)


        @block.vector
        def _(vector):
            for c in range(NCHUNK):
                vector.wait_ge(x_sems[c], 16)
                vector.wait_ge(g_sems[c], 16)
                nc.vector.tensor_tensor(
                    out=diff_t[:, cols(c)],
                    in0=x_t[:, cols(c)],
                    in1=cg_t[:, cols(c)],
                    op=mybir.AluOpType.subtract,
                ).then_inc(v_sem, 1)
            # DVE is pipelined, so same-engine RAW chains need explicit waits.
            vector.wait_ge(a_sem, NCHUNK)
            nc.vector.reduce_sum(
                out=dist_t[:], in_=dist4_t[:], axis=mybir.AxisListType.X
            ).then_inc(v_sem, 1)
            vector.wait_ge(v_sem, NCHUNK + 1)
            nc.vector.tensor_scalar(
                out=dist_t[:],
                in0=dist_t[:],
                scalar1=1e-12,
                scalar2=1e12,
                op0=mybir.AluOpType.max,
                op1=mybir.AluOpType.min,
            ).then_inc(r_sem, 1)
            vector.wait_ge(mm_sem, 1)
            nc.vector.tensor_copy(out=res_t[:], in_=psum_t[:]).then_inc(done_sem, 1)

        @block.scalar
        def _(scalar):
            for c in range(NCHUNK):
                scalar.wait_ge(v_sem, c + 1)
                nc.scalar.activation(
                    out=x_t[:, cols(c)],
                    in_=diff_t[:, cols(c)],
                    func=mybir.ActivationFunctionType.Square,
                    accum_out=dist4_t[:, c : c + 1],
                ).then_inc(a_sem, 1)


        @block.tensor
        def _(tensor):
            tensor.wait_ge(r_sem, 1)
            nc.tensor.matmul(
                out=psum_t[:], lhsT=dist_t[:], rhs=ones, start=True, stop=True
            ).then_inc(mm_sem, 1)

        # Re-execution safety: the same loaded NEFF runs many times, so all
        # kernel sems must end at 0. Builtin all-engine barrier (self-
        # resetting gather/release sems) orders every engine's updates
        # before Pool drains DMA state and zeroes the kernel semaphores.
        nc.all_engine_barrier()
        nc.gpsimd.dma_reset(sem_range)
        nc.gpsimd.sem_clear(sem_range)

    return nc


def _get_nc():
    if "nc" not in _CACHE:
        _CACHE["nc"] = _build()
    return _CACHE["nc"]


def kernel(x: np.ndarray, labels: np.ndarray, centers: np.ndarray) -> np.ndarray:
    x = np.ascontiguousarray(np.asarray(x, dtype=np.float32))
    centers = np.ascontiguousarray(np.asarray(centers, dtype=np.float32))
    lab = np.asarray(labels).astype(np.int32).reshape(B, 1)

    nc = _get_nc()
    in_maps = []
    for c in range(N_CORES):
        sl = slice(c * P, (c + 1) * P)
        in_maps.append(
            {
                "x": np.ascontiguousarray(x[sl]),
                "labels": np.ascontiguousarray(lab[sl]),
                "centers": centers,
            }
        )
    res = run_bass_kernel_spmd(nc, in_maps, list(range(N_CORES)))

    total = 0.0
    for c in range(N_CORES):
        total += float(res.results[c]["out"][0, 0])
    total += (B * C - B) * 1e-12
    return np.float32(total / B)
